# revision 10
# baseline (speedup 1.0000x reference)
"""Fused single-launch Trainium2 kernel for nn_Attention2.

Math (folded):
    s    = colsum(Z_l) - colsum(Z_g) = colsum(D),  D = Z_l - Z_g   # [512]
    u    = W.T @ s                                                  # [512]
    c    = b . s
    d    = Z_o @ u + c                                              # [N]
    out  = Z_g + sigmoid(d) * D

Single SPMD launch per pass:
  Phase 1: stream ZLG chunks (Z_l/Z_g interleaved per 256-row chunk, 1MB
           DMAs alternating the two HWDGE rings), D = Z_l - Z_g in fp32
           (DVE), colsum(D) via fp32 ones-matmuls into PSUM (PE), cast D
           and Z_g to bf16 into a big SBUF cache (ACT).
  AllReduce s across the 8 cores (2KB, on-device collective).
  u-setup: PE transposes of s, u = W.T @ s + broadcast on PE, c = b.s.
  Phase 2: stream Z_o fp32; prod = Z_o * u (DVE) into PSUM; d = rowsum
           via ACT Identity+accum_out with bias c/512 (ScalarE);
           sigmoid (ScalarE); blend out = D*sig + Z_g via
           scalar_tensor_tensor (DVE) from the bf16 cache; bf16 out.
           Chunks that did not fit in the cache re-read ZLG fp32.

Rows padded host-side to 12544/core (49 chunks x 256 rows) with zeros.
"""

import numpy as np

import concourse.bacc as bacc
import concourse.mybir as mybir
import concourse.tile as tile
from concourse.bass_utils import run_bass_kernel_spmd

N_CORES = 8
N_TOTAL = 100000
CH = 512
P = 128
ROWS_PER_CHUNK = 256          # 2 row-tiles of 128
N_CHUNKS = 49
SHARD_PAD = N_CHUNKS * ROWS_PER_CHUNK  # 12544
SHARD = N_TOTAL // N_CORES    # 12500 real rows per core

f32 = mybir.dt.float32
bf16 = mybir.dt.bfloat16

N_CACHED = 35


def build(rep=1, n_cached=N_CACHED, in_bufs=8, out_bufs=3, n_cores=N_CORES,
          out_on_gpsimd=True, cast_split=True, blend_sg=False):
    AF = mybir.ActivationFunctionType
    mult = mybir.AluOpType.mult
    add = mybir.AluOpType.add

    nc = bacc.Bacc(
        "TRN2",
        target_bir_lowering=False,
        debug=False,
        enable_asserts=False,
        num_devices=n_cores,
    )
    zlg_d = nc.dram_tensor("ZLG", [2 * SHARD_PAD, CH], f32, kind="ExternalInput")
    zo_d = nc.dram_tensor("Z_o", [SHARD_PAD, CH], f32, kind="ExternalInput")
    w_d = nc.dram_tensor("W", [CH, CH], f32, kind="ExternalInput")
    b_d = nc.dram_tensor("b", [1, CH], f32, kind="ExternalInput")
    out_d = nc.dram_tensor("out", [SHARD_PAD, CH], bf16, kind="ExternalOutput")

    with tile.TileContext(nc) as tc:
        with (
            tc.tile_pool(name="singles", bufs=1) as singles,
            tc.tile_pool(name="cachep", bufs=1) as cachep,
            tc.tile_pool(name="stream", bufs=in_bufs) as stream,
            tc.tile_pool(name="dpool", bufs=2) as dpool,
            tc.tile_pool(name="opool", bufs=out_bufs) as opool,
            tc.tile_pool(name="wpool", bufs=1) as wpool,
            tc.tile_pool(name="small1", bufs=1) as small1,
            tc.tile_pool(name="smalln", bufs=4) as smalln,
            tc.tile_pool(name="ps_acc", bufs=1, space="PSUM") as ps_acc,
            tc.tile_pool(name="ps_prod", bufs=1, space="PSUM") as ps_prod,
            tc.tile_pool(name="ps_dummy", bufs=1, space="PSUM") as ps_dummy,
            tc.tile_pool(name="ps_u", bufs=1, space="PSUM") as ps_u,
            tc.tile_pool(name="ps_misc", bufs=1, space="PSUM") as ps_misc,
            tc.tile_pool(name="dram", bufs=2, space="DRAM") as drams,
        ):
            ones_col = singles.tile([P, 1], f32)
            nc.vector.memset(ones_col[:], 1.0)
            ones_row = singles.tile([1, P], f32)
            nc.vector.memset(ones_row[:], 1.0)
            b_sb = singles.tile([1, CH], f32)
            nc.sync.dma_start(b_sb[:], b_d[:, :])

            # big bf16 caches for D and Z_g (n_cached chunks x 1024 f-elems)
            cache_D = cachep.tile([P, n_cached * 1024], bf16)
            cache_G = cachep.tile([P, n_cached * 1024], bf16)

            pools = dict(
                stream=stream, dpool=dpool, opool=opool,
                wpool=wpool, small1=small1, smalln=smalln, ps_acc=ps_acc,
                ps_prod=ps_prod, ps_dummy=ps_dummy, ps_u=ps_u,
                ps_misc=ps_misc, drams=drams,
            )
            consts = dict(
                ones_col=ones_col, ones_row=ones_row, b_sb=b_sb,
                cache_D=cache_D, cache_G=cache_G,
            )
            for _ in range(rep):
                _one_pass(nc, AF, mult, add, zlg_d, zo_d, w_d, out_d,
                          pools, consts, n_cached, n_cores,
                          out_on_gpsimd, cast_split, blend_sg)
    nc.compile()
    return nc


def _one_pass(nc, AF, mult, add, zlg_d, zo_d, w_d, out_d,
              pools, consts, n_cached, n_cores,
              out_on_gpsimd=False, cast_split=False, blend_sg=False):
    stream = pools["stream"]
    dpool = pools["dpool"]
    opool = pools["opool"]
    wpool = pools["wpool"]
    small1 = pools["small1"]
    smalln = pools["smalln"]
    ps_acc = pools["ps_acc"]
    ps_prod = pools["ps_prod"]
    ps_dummy = pools["ps_dummy"]
    ps_u = pools["ps_u"]
    ps_misc = pools["ps_misc"]
    drams = pools["drams"]
    ones_col = consts["ones_col"]
    ones_row = consts["ones_row"]
    b_sb = consts["b_sb"]
    cache_D = consts["cache_D"]
    cache_G = consts["cache_G"]

    # ---------------- Phase 1: colsum(D) + bf16 cache fill ----------------
    ps_s = ps_acc.tile([1, CH], f32, tag="ps_s")
    for k in range(N_CHUNKS):
        r0 = 512 * k
        zl_t = stream.tile([P, 2 * CH], f32, tag="in")
        zg_t = stream.tile([P, 2 * CH], f32, tag="in")
        dma_a = nc.sync if (k % 2 == 0) else nc.scalar
        dma_b = nc.scalar if (k % 2 == 0) else nc.sync
        dma_a.dma_start(
            zl_t[:].rearrange("p (j c) -> p j c", c=CH),
            zlg_d[r0 : r0 + 256].rearrange("(j p) c -> p j c", p=P),
        )
        dma_b.dma_start(
            zg_t[:].rearrange("p (j c) -> p j c", c=CH),
            zlg_d[r0 + 256 : r0 + 512].rearrange("(j p) c -> p j c", p=P),
        )
        dfp = dpool.tile([P, 2 * CH], f32, tag="dfp")
        nc.vector.tensor_sub(dfp[:], zl_t[:], zg_t[:])
        for j in range(2):
            nc.tensor.matmul(
                ps_s[:],
                ones_col[:],
                dfp[:, j * CH : (j + 1) * CH],
                start=(k == 0 and j == 0),
                stop=(k == N_CHUNKS - 1 and j == 1),
            )
        if k < n_cached:
            co = k * 1024
            if cast_split:
                nc.vector.tensor_copy(cache_D[:, co : co + 1024], dfp[:])
            else:
                nc.scalar.copy(cache_D[:, co : co + 1024], dfp[:])
            nc.scalar.copy(cache_G[:, co : co + 1024], zg_t[:])

    s_sb = small1.tile([1, CH], f32, tag="s_sb")
    nc.vector.tensor_copy(s_sb[:], ps_s[:])

    # ---------------- AllReduce s ----------------
    cc_in = drams.tile([1, CH], f32, tag="cc_in")
    cc_out = drams.tile([1, CH], f32, tag="cc_out", addr_space="Shared")
    nc.sync.dma_start(cc_in[:], s_sb[:])
    nc.gpsimd.collective_compute(
        "AllReduce",
        mybir.AluOpType.add,
        replica_groups=[list(range(n_cores))],
        ins=[cc_in[:]],
        outs=[cc_out[:]],
    )
    s_ar = small1.tile([1, CH], f32, tag="s_ar")
    nc.sync.dma_start(s_ar[:], cc_out[:])

    # ---------------- u = W.T @ s (broadcast); c = b . s ----------------
    sT = small1.tile([P, 4], f32, tag="sT")
    for cix in range(4):
        tp = ps_misc.tile([P, 1], f32, tag="tp")
        nc.tensor.transpose(
            tp[:], s_ar[0:1, cix * P : (cix + 1) * P], ones_row[0:1, 0:1]
        )
        nc.vector.tensor_copy(sT[:, cix : cix + 1], tp[:])
    u_ps = ps_misc.tile([1, CH], f32, tag="ups")
    for cix in range(4):
        w_c = stream.tile([P, CH], f32, tag="in")
        nc.scalar.dma_start(w_c[:], w_d[cix * P : (cix + 1) * P])
        nc.tensor.matmul(
            u_ps[:], sT[:, cix : cix + 1], w_c[:],
            start=(cix == 0), stop=(cix == 3),
        )
    u_sb = small1.tile([1, CH], f32, tag="u_sb")
    nc.vector.tensor_copy(u_sb[:], u_ps[:])
    ub_ps = ps_u.tile([P, CH], f32, tag="ub")
    nc.tensor.matmul(ub_ps[:], ones_row[:], u_sb[:], start=True, stop=True)
    u_bsb = small1.tile([P, 2 * CH], f32, tag="u_bsb")
    nc.vector.tensor_copy(u_bsb[:, 0:CH], ub_ps[:])
    nc.vector.tensor_copy(u_bsb[:, CH : 2 * CH], ub_ps[:])

    bs_sb = small1.tile([1, CH], f32, tag="bs_sb")
    nc.vector.tensor_mul(bs_sb[:], b_sb[:], s_ar[:])
    bs_ps = ps_misc.tile([P, CH], f32, tag="cps")
    nc.tensor.matmul(bs_ps[:], ones_row[:], bs_sb[:], start=True, stop=True)
    c_div = small1.tile([P, 1], f32, tag="c_div")
    nc.vector.tensor_reduce(
        c_div[:], bs_ps[:], axis=mybir.AxisListType.X, op=add
    )
    # bias applied per element inside the ACT reduce: c/CH each, CH times
    nc.vector.tensor_scalar_mul(c_div[:], c_div[:], 1.0 / CH)

    # ---------------- Phase 2: d, sigmoid, blend, store ----------------
    for k in range(N_CHUNKS):
        r0 = 256 * k
        zo = stream.tile([P, 2 * CH], f32, tag="in")
        dma_eng = nc.sync if (k % 2 == 0) else nc.scalar
        dma_eng.dma_start(
            zo[:].rearrange("p (j c) -> p j c", c=CH),
            zo_d[r0 : r0 + 256].rearrange("(j p) c -> p j c", p=P),
        )
        prod = ps_prod.tile([P, 2 * CH], f32, tag="prod")
        nc.vector.tensor_mul(prod[:], zo[:], u_bsb[:])
        d_t = smalln.tile([P, 2], f32, tag="d_t")
        for j in range(2):
            dummy = ps_dummy.tile([P, CH], f32, tag="dummy")
            nc.scalar.activation(
                dummy[:], prod[:, j * CH : (j + 1) * CH], AF.Identity,
                bias=c_div[:, 0:1], scale=1.0,
                accum_out=d_t[:, j : j + 1],
            )
        frac = smalln.tile([P, 2], f32, tag="frac")
        nc.scalar.activation(frac[:], d_t[:], AF.Sigmoid, bias=0.0, scale=1.0)

        ot = opool.tile([P, 2 * CH], bf16, tag="ot")
        if k < n_cached:
            co = k * 1024
            if blend_sg:
                t_sc = dpool.tile([P, 2 * CH], bf16, tag="tsc")
                for j in range(2):
                    nc.scalar.mul(
                        t_sc[:, j * CH : (j + 1) * CH],
                        cache_D[:, co + j * CH : co + (j + 1) * CH],
                        frac[:, j : j + 1],
                    )
                nc.gpsimd.tensor_add(
                    ot[:], t_sc[:], cache_G[:, co : co + 1024]
                )
            else:
                for j in range(2):
                    nc.vector.scalar_tensor_tensor(
                        out=ot[:, j * CH : (j + 1) * CH],
                        in0=cache_D[:, co + j * CH : co + (j + 1) * CH],
                        scalar=frac[:, j : j + 1],
                        in1=cache_G[:, co + j * CH : co + (j + 1) * CH],
                        op0=mult,
                        op1=add,
                    )
        else:
            zl2 = stream.tile([P, 2 * CH], f32, tag="in")
            zg2 = stream.tile([P, 2 * CH], f32, tag="in")
            dma_a2 = nc.scalar if (k % 2 == 0) else nc.sync
            dma_b2 = nc.sync if (k % 2 == 0) else nc.scalar
            dma_a2.dma_start(
                zl2[:].rearrange("p (j c) -> p j c", c=CH),
                zlg_d[2 * r0 : 2 * r0 + 256].rearrange("(j p) c -> p j c", p=P),
            )
            dma_b2.dma_start(
                zg2[:].rearrange("p (j c) -> p j c", c=CH),
                zlg_d[2 * r0 + 256 : 2 * r0 + 512].rearrange(
                    "(j p) c -> p j c", p=P
                ),
            )
            dfp2 = dpool.tile([P, 2 * CH], f32, tag="dfp")
            nc.vector.tensor_sub(dfp2[:], zl2[:], zg2[:])
            if blend_sg:
                t_sc2 = dpool.tile([P, 2 * CH], bf16, tag="tsc")
                for j in range(2):
                    nc.scalar.mul(
                        t_sc2[:, j * CH : (j + 1) * CH],
                        dfp2[:, j * CH : (j + 1) * CH],
                        frac[:, j : j + 1],
                    )
                nc.gpsimd.tensor_add(ot[:], t_sc2[:], zg2[:])
            else:
                for j in range(2):
                    nc.vector.scalar_tensor_tensor(
                        out=ot[:, j * CH : (j + 1) * CH],
                        in0=dfp2[:, j * CH : (j + 1) * CH],
                        scalar=frac[:, j : j + 1],
                        in1=zg2[:, j * CH : (j + 1) * CH],
                        op0=mult,
                        op1=add,
                    )
        if out_on_gpsimd:
            dma_eng3 = nc.gpsimd
        else:
            dma_eng3 = nc.scalar if (k % 2 == 0) else nc.sync
        dma_eng3.dma_start(
            out_d[r0 : r0 + 256].rearrange("(j p) c -> p j c", p=P),
            ot[:].rearrange("p (j c) -> p j c", c=CH),
        )


_CACHE = {}


def _prep_core_inputs(Z_o, Z_l, Z_g, W, b, core):
    lo = core * SHARD
    hi = lo + SHARD
    zl = np.zeros((SHARD_PAD, CH), dtype=np.float32)
    zl[:SHARD] = Z_l[lo:hi]
    zg = np.zeros((SHARD_PAD, CH), dtype=np.float32)
    zg[:SHARD] = Z_g[lo:hi]
    zlg = np.concatenate(
        [zl.reshape(N_CHUNKS, ROWS_PER_CHUNK, CH),
         zg.reshape(N_CHUNKS, ROWS_PER_CHUNK, CH)],
        axis=1,
    ).reshape(2 * SHARD_PAD, CH)
    zo = np.zeros((SHARD_PAD, CH), dtype=np.float32)
    zo[:SHARD] = Z_o[lo:hi]
    return {
        "ZLG": np.ascontiguousarray(zlg),
        "Z_o": np.ascontiguousarray(zo),
        "W": np.ascontiguousarray(W),
        "b": np.ascontiguousarray(b.reshape(1, CH)),
    }


def kernel(Z_o, Z_l, Z_g, W, b):
    Z_o = np.asarray(Z_o, dtype=np.float32)
    Z_l = np.asarray(Z_l, dtype=np.float32)
    Z_g = np.asarray(Z_g, dtype=np.float32)
    W = np.asarray(W, dtype=np.float32)
    b = np.asarray(b, dtype=np.float32)
    if "nc" not in _CACHE:
        _CACHE["nc"] = build()
    nc = _CACHE["nc"]
    maps = [
        _prep_core_inputs(Z_o, Z_l, Z_g, W, b, core) for core in range(N_CORES)
    ]
    res = run_bass_kernel_spmd(nc, maps, core_ids=list(range(N_CORES)))
    outs = [
        np.asarray(r["out"][:SHARD]).astype(np.float32) for r in res.results
    ]
    return np.concatenate(outs, axis=0)


# revision 11
# speedup vs baseline: 1.2568x; 1.2568x over previous
"""Fused single-launch Trainium2 kernel for nn_Attention2.

Math (folded):
    s    = colsum(Z_l) - colsum(Z_g) = colsum(D),  D = Z_l - Z_g   # [512]
    u    = W.T @ s                                                  # [512]
    c    = b . s
    d    = Z_o @ u + c                                              # [N]
    out  = Z_g + sigmoid(d) * D

Single SPMD launch per pass:
  Phase 1: stream ZLG chunks (Z_l/Z_g blocked per 256-row chunk; two
           512KB DMAs per chunk alternating the two HWDGE rings),
           D = Z_l - Z_g in fp32 (DVE), colsum(D) via fp32 ones-matmuls
           accumulated in PSUM (PE), cast D and Z_g to bf16 into big
           SBUF cache tiles (DVE/ACT per cast_split).
  AllReduce s across the 8 cores (2KB, on-device collective).
  u-setup: PE transposes of s, u = W.T @ s + broadcast on PE, c = b.s.
  Phase 2: stream Z_o fp32; prod = Z_o * u (DVE) into PSUM; d = rowsum
           via ACT Identity+accum_out with bias c/512 (ScalarE);
           sigmoid (ScalarE); blend out = D*sig(d) + Z_g via fused
           scalar_tensor_tensor (DVE) from the bf16 cache; bf16 output
           written on the SWDGE (gpsimd) ring. Chunks that did not fit
           in the cache re-read ZLG fp32 and blend from fp32.

Rows padded host-side to 12544/core (49 chunks x 256 rows) with zeros;
per-core HBM traffic ~103MB vs ~154MB for the two-launch baseline, and
the d-path runs fused DVE/ACT ops instead of mul+reduce+two scalar muls.

Notes from bring-up (this axon/bass stack):
  - nc.vector.tensor_tensor_reduce wedges the device (NRT status 101);
    the ACT accum_out path replaces it.
  - scalar_tensor_tensor is DVE-only (Pool fails the ISA opcode check).
  - PE matmul/transpose with K=1 works; an N=1 matmul (rhs [1,1]) fails
    at runtime - c is broadcast via ones-matmul + free-dim reduce.
"""

import numpy as np

import concourse.bacc as bacc
import concourse.mybir as mybir
import concourse.tile as tile
from concourse.bass_utils import run_bass_kernel_spmd

N_CORES = 8
N_TOTAL = 100000
CH = 512
P = 128
ROWS_PER_CHUNK = 256          # 2 row-tiles of 128
N_CHUNKS = 49
SHARD_PAD = N_CHUNKS * ROWS_PER_CHUNK  # 12544
SHARD = N_TOTAL // N_CORES    # 12500 real rows per core

f32 = mybir.dt.float32
bf16 = mybir.dt.bfloat16

N_CACHED = 35


def build(rep=1, n_cached=N_CACHED, in_bufs=8, out_bufs=3, n_cores=N_CORES,
          out_on_gpsimd=True, cast_split=True, blend_sg=False):
    AF = mybir.ActivationFunctionType
    mult = mybir.AluOpType.mult
    add = mybir.AluOpType.add

    nc = bacc.Bacc(
        "TRN2",
        target_bir_lowering=False,
        debug=False,
        enable_asserts=False,
        num_devices=n_cores,
    )
    zlg_d = nc.dram_tensor("ZLG", [2 * SHARD_PAD, CH], f32, kind="ExternalInput")
    zo_d = nc.dram_tensor("Z_o", [SHARD_PAD, CH], f32, kind="ExternalInput")
    w_d = nc.dram_tensor("W", [CH, CH], f32, kind="ExternalInput")
    b_d = nc.dram_tensor("b", [1, CH], f32, kind="ExternalInput")
    out_d = nc.dram_tensor("out", [SHARD_PAD, CH], bf16, kind="ExternalOutput")

    with tile.TileContext(nc) as tc:
        with (
            tc.tile_pool(name="singles", bufs=1) as singles,
            tc.tile_pool(name="cachep", bufs=1) as cachep,
            tc.tile_pool(name="stream", bufs=in_bufs) as stream,
            tc.tile_pool(name="dpool", bufs=2) as dpool,
            tc.tile_pool(name="opool", bufs=out_bufs) as opool,
            tc.tile_pool(name="wpool", bufs=1) as wpool,
            tc.tile_pool(name="small1", bufs=1) as small1,
            tc.tile_pool(name="smalln", bufs=4) as smalln,
            tc.tile_pool(name="ps_acc", bufs=1, space="PSUM") as ps_acc,
            tc.tile_pool(name="ps_prod", bufs=1, space="PSUM") as ps_prod,
            tc.tile_pool(name="ps_dummy", bufs=1, space="PSUM") as ps_dummy,
            tc.tile_pool(name="ps_u", bufs=1, space="PSUM") as ps_u,
            tc.tile_pool(name="ps_misc", bufs=1, space="PSUM") as ps_misc,
            tc.tile_pool(name="dram", bufs=2, space="DRAM") as drams,
        ):
            ones_col = singles.tile([P, 1], f32)
            nc.vector.memset(ones_col[:], 1.0)
            ones_row = singles.tile([1, P], f32)
            nc.vector.memset(ones_row[:], 1.0)
            b_sb = singles.tile([1, CH], f32)
            nc.sync.dma_start(b_sb[:], b_d[:, :])

            # big bf16 caches for D and Z_g (n_cached chunks x 1024 f-elems)
            cache_D = cachep.tile([P, n_cached * 1024], bf16)
            cache_G = cachep.tile([P, n_cached * 1024], bf16)

            pools = dict(
                stream=stream, dpool=dpool, opool=opool,
                wpool=wpool, small1=small1, smalln=smalln, ps_acc=ps_acc,
                ps_prod=ps_prod, ps_dummy=ps_dummy, ps_u=ps_u,
                ps_misc=ps_misc, drams=drams,
            )
            consts = dict(
                ones_col=ones_col, ones_row=ones_row, b_sb=b_sb,
                cache_D=cache_D, cache_G=cache_G,
            )
            for _ in range(rep):
                _one_pass(nc, AF, mult, add, zlg_d, zo_d, w_d, out_d,
                          pools, consts, n_cached, n_cores,
                          out_on_gpsimd, cast_split, blend_sg)
    nc.compile()
    return nc


def _one_pass(nc, AF, mult, add, zlg_d, zo_d, w_d, out_d,
              pools, consts, n_cached, n_cores,
              out_on_gpsimd=False, cast_split=False, blend_sg=False):
    stream = pools["stream"]
    dpool = pools["dpool"]
    opool = pools["opool"]
    wpool = pools["wpool"]
    small1 = pools["small1"]
    smalln = pools["smalln"]
    ps_acc = pools["ps_acc"]
    ps_prod = pools["ps_prod"]
    ps_dummy = pools["ps_dummy"]
    ps_u = pools["ps_u"]
    ps_misc = pools["ps_misc"]
    drams = pools["drams"]
    ones_col = consts["ones_col"]
    ones_row = consts["ones_row"]
    b_sb = consts["b_sb"]
    cache_D = consts["cache_D"]
    cache_G = consts["cache_G"]

    # ---------------- Phase 1: colsum(D) + bf16 cache fill ----------------
    ps_s = ps_acc.tile([1, CH], f32, tag="ps_s")
    for k in range(N_CHUNKS):
        r0 = 512 * k
        zl_t = stream.tile([P, 2 * CH], f32, tag="in")
        zg_t = stream.tile([P, 2 * CH], f32, tag="in")
        dma_a = nc.sync if (k % 2 == 0) else nc.scalar
        dma_b = nc.scalar if (k % 2 == 0) else nc.sync
        dma_a.dma_start(
            zl_t[:].rearrange("p (j c) -> p j c", c=CH),
            zlg_d[r0 : r0 + 256].rearrange("(j p) c -> p j c", p=P),
        )
        dma_b.dma_start(
            zg_t[:].rearrange("p (j c) -> p j c", c=CH),
            zlg_d[r0 + 256 : r0 + 512].rearrange("(j p) c -> p j c", p=P),
        )
        dfp = dpool.tile([P, 2 * CH], f32, tag="dfp")
        nc.vector.tensor_sub(dfp[:], zl_t[:], zg_t[:])
        for j in range(2):
            nc.tensor.matmul(
                ps_s[:],
                ones_col[:],
                dfp[:, j * CH : (j + 1) * CH],
                start=(k == 0 and j == 0),
                stop=(k == N_CHUNKS - 1 and j == 1),
            )
        if k < n_cached:
            co = k * 1024
            if cast_split:
                nc.vector.tensor_copy(cache_D[:, co : co + 1024], dfp[:])
            else:
                nc.scalar.copy(cache_D[:, co : co + 1024], dfp[:])
            nc.scalar.copy(cache_G[:, co : co + 1024], zg_t[:])

    s_sb = small1.tile([1, CH], f32, tag="s_sb")
    nc.vector.tensor_copy(s_sb[:], ps_s[:])

    # ---------------- AllReduce s ----------------
    cc_in = drams.tile([1, CH], f32, tag="cc_in")
    cc_out = drams.tile([1, CH], f32, tag="cc_out", addr_space="Shared")
    nc.sync.dma_start(cc_in[:], s_sb[:])
    nc.gpsimd.collective_compute(
        "AllReduce",
        mybir.AluOpType.add,
        replica_groups=[list(range(n_cores))],
        ins=[cc_in[:]],
        outs=[cc_out[:]],
    )
    s_ar = small1.tile([1, CH], f32, tag="s_ar")
    nc.sync.dma_start(s_ar[:], cc_out[:])

    # ---------------- u = W.T @ s (broadcast); c = b . s ----------------
    sT = small1.tile([P, 4], f32, tag="sT")
    for cix in range(4):
        tp = ps_misc.tile([P, 1], f32, tag="tp")
        nc.tensor.transpose(
            tp[:], s_ar[0:1, cix * P : (cix + 1) * P], ones_row[0:1, 0:1]
        )
        nc.vector.tensor_copy(sT[:, cix : cix + 1], tp[:])
    u_ps = ps_misc.tile([1, CH], f32, tag="ups")
    for cix in range(4):
        w_c = stream.tile([P, CH], f32, tag="in")
        nc.scalar.dma_start(w_c[:], w_d[cix * P : (cix + 1) * P])
        nc.tensor.matmul(
            u_ps[:], sT[:, cix : cix + 1], w_c[:],
            start=(cix == 0), stop=(cix == 3),
        )
    u_sb = small1.tile([1, CH], f32, tag="u_sb")
    nc.vector.tensor_copy(u_sb[:], u_ps[:])
    ub_ps = ps_u.tile([P, CH], f32, tag="ub")
    nc.tensor.matmul(ub_ps[:], ones_row[:], u_sb[:], start=True, stop=True)
    u_bsb = small1.tile([P, 2 * CH], f32, tag="u_bsb")
    nc.vector.tensor_copy(u_bsb[:, 0:CH], ub_ps[:])
    nc.vector.tensor_copy(u_bsb[:, CH : 2 * CH], ub_ps[:])

    bs_sb = small1.tile([1, CH], f32, tag="bs_sb")
    nc.vector.tensor_mul(bs_sb[:], b_sb[:], s_ar[:])
    bs_ps = ps_misc.tile([P, CH], f32, tag="cps")
    nc.tensor.matmul(bs_ps[:], ones_row[:], bs_sb[:], start=True, stop=True)
    c_div = small1.tile([P, 1], f32, tag="c_div")
    nc.vector.tensor_reduce(
        c_div[:], bs_ps[:], axis=mybir.AxisListType.X, op=add
    )
    # bias applied per element inside the ACT reduce: c/CH each, CH times
    nc.vector.tensor_scalar_mul(c_div[:], c_div[:], 1.0 / CH)

    # ---------------- Phase 2: d, sigmoid, blend, store ----------------
    for k in range(N_CHUNKS):
        r0 = 256 * k
        zo = stream.tile([P, 2 * CH], f32, tag="in")
        dma_eng = nc.sync if (k % 2 == 0) else nc.scalar
        dma_eng.dma_start(
            zo[:].rearrange("p (j c) -> p j c", c=CH),
            zo_d[r0 : r0 + 256].rearrange("(j p) c -> p j c", p=P),
        )
        prod = ps_prod.tile([P, 2 * CH], f32, tag="prod")
        nc.vector.tensor_mul(prod[:], zo[:], u_bsb[:])
        d_t = smalln.tile([P, 2], f32, tag="d_t")
        for j in range(2):
            dummy = ps_dummy.tile([P, CH], f32, tag="dummy")
            nc.scalar.activation(
                dummy[:], prod[:, j * CH : (j + 1) * CH], AF.Identity,
                bias=c_div[:, 0:1], scale=1.0,
                accum_out=d_t[:, j : j + 1],
            )
        frac = smalln.tile([P, 2], f32, tag="frac")
        nc.scalar.activation(frac[:], d_t[:], AF.Sigmoid, bias=0.0, scale=1.0)

        ot = opool.tile([P, 2 * CH], bf16, tag="ot")
        if k < n_cached:
            co = k * 1024
            if blend_sg:
                t_sc = dpool.tile([P, 2 * CH], bf16, tag="tsc")
                for j in range(2):
                    nc.scalar.mul(
                        t_sc[:, j * CH : (j + 1) * CH],
                        cache_D[:, co + j * CH : co + (j + 1) * CH],
                        frac[:, j : j + 1],
                    )
                nc.gpsimd.tensor_add(
                    ot[:], t_sc[:], cache_G[:, co : co + 1024]
                )
            else:
                for j in range(2):
                    nc.vector.scalar_tensor_tensor(
                        out=ot[:, j * CH : (j + 1) * CH],
                        in0=cache_D[:, co + j * CH : co + (j + 1) * CH],
                        scalar=frac[:, j : j + 1],
                        in1=cache_G[:, co + j * CH : co + (j + 1) * CH],
                        op0=mult,
                        op1=add,
                    )
        else:
            zl2 = stream.tile([P, 2 * CH], f32, tag="in")
            zg2 = stream.tile([P, 2 * CH], f32, tag="in")
            dma_a2 = nc.scalar if (k % 2 == 0) else nc.sync
            dma_b2 = nc.sync if (k % 2 == 0) else nc.scalar
            dma_a2.dma_start(
                zl2[:].rearrange("p (j c) -> p j c", c=CH),
                zlg_d[2 * r0 : 2 * r0 + 256].rearrange("(j p) c -> p j c", p=P),
            )
            dma_b2.dma_start(
                zg2[:].rearrange("p (j c) -> p j c", c=CH),
                zlg_d[2 * r0 + 256 : 2 * r0 + 512].rearrange(
                    "(j p) c -> p j c", p=P
                ),
            )
            dfp2 = dpool.tile([P, 2 * CH], f32, tag="dfp")
            nc.vector.tensor_sub(dfp2[:], zl2[:], zg2[:])
            if blend_sg:
                t_sc2 = dpool.tile([P, 2 * CH], bf16, tag="tsc")
                for j in range(2):
                    nc.scalar.mul(
                        t_sc2[:, j * CH : (j + 1) * CH],
                        dfp2[:, j * CH : (j + 1) * CH],
                        frac[:, j : j + 1],
                    )
                nc.gpsimd.tensor_add(ot[:], t_sc2[:], zg2[:])
            else:
                for j in range(2):
                    nc.vector.scalar_tensor_tensor(
                        out=ot[:, j * CH : (j + 1) * CH],
                        in0=dfp2[:, j * CH : (j + 1) * CH],
                        scalar=frac[:, j : j + 1],
                        in1=zg2[:, j * CH : (j + 1) * CH],
                        op0=mult,
                        op1=add,
                    )
        if out_on_gpsimd:
            dma_eng3 = nc.gpsimd
        else:
            dma_eng3 = nc.scalar if (k % 2 == 0) else nc.sync
        dma_eng3.dma_start(
            out_d[r0 : r0 + 256].rearrange("(j p) c -> p j c", p=P),
            ot[:].rearrange("p (j c) -> p j c", c=CH),
        )


_CACHE = {}


def _prep_core_inputs(Z_o, Z_l, Z_g, W, b, core):
    lo = core * SHARD
    hi = lo + SHARD
    zl = np.zeros((SHARD_PAD, CH), dtype=np.float32)
    zl[:SHARD] = Z_l[lo:hi]
    zg = np.zeros((SHARD_PAD, CH), dtype=np.float32)
    zg[:SHARD] = Z_g[lo:hi]
    zlg = np.concatenate(
        [zl.reshape(N_CHUNKS, ROWS_PER_CHUNK, CH),
         zg.reshape(N_CHUNKS, ROWS_PER_CHUNK, CH)],
        axis=1,
    ).reshape(2 * SHARD_PAD, CH)
    zo = np.zeros((SHARD_PAD, CH), dtype=np.float32)
    zo[:SHARD] = Z_o[lo:hi]
    return {
        "ZLG": np.ascontiguousarray(zlg),
        "Z_o": np.ascontiguousarray(zo),
        "W": np.ascontiguousarray(W),
        "b": np.ascontiguousarray(b.reshape(1, CH)),
    }


def kernel(Z_o, Z_l, Z_g, W, b):
    Z_o = np.asarray(Z_o, dtype=np.float32)
    Z_l = np.asarray(Z_l, dtype=np.float32)
    Z_g = np.asarray(Z_g, dtype=np.float32)
    W = np.asarray(W, dtype=np.float32)
    b = np.asarray(b, dtype=np.float32)
    if "nc" not in _CACHE:
        _CACHE["nc"] = build()
    nc = _CACHE["nc"]
    maps = [
        _prep_core_inputs(Z_o, Z_l, Z_g, W, b, core) for core in range(N_CORES)
    ]
    res = run_bass_kernel_spmd(nc, maps, core_ids=list(range(N_CORES)))
    outs = [
        np.asarray(r["out"][:SHARD]).astype(np.float32) for r in res.results
    ]
    return np.concatenate(outs, axis=0)
